# revision 53
# baseline (speedup 1.0000x reference)
"""Trainium2 Bass kernel: single-head causal attention (B=16, T=2048, C=1024, HD=64).

Data-parallel over batch across 8 NeuronCores (2 batches/core), weights
replicated. Each core computes, per batch:
    q = x @ Wq, k = x @ Wk, v = x @ Wv            (via transposed layouts)
    scores[t, s] = k[t] . q[s] / sqrt(C)          (computed transposed: St[s, t])
    causal mask (keep s <= t), softmax over s, out[t] = sum_s w[t, s] v[s]

Compute dtype is bf16 on the TensorEngine (PSUM accumulation in fp32, final
softmax division in fp32); rel-err vs the fp32 reference is ~4e-3.

Layout strategy (v3):
  - x arrives as one DMA per 512-t chunk [128, 4tt, 1024c], is cast to bf16
    (DVE), then transposed 128x128-at-a-time on the PE as REGULAR bf16
    matmuls against a bf16 identity (LDW = x tile, stream identity, N=128) -
    much faster than transpose-mode; 4 tiles pack into one PSUM bank and one
    DVE copy moves them to xT [128 c_inner, ncc, t].
  - q/k come from one stacked [Wq|Wk] projection into qkT (q rows 0:64, k
    rows 64:128); a swapped duplicate qkT2 (k lo, q hi) lets the K=64 score
    matmuls run two-at-a-time in separate PE row groups.
  - v is projected into vT [64h, s] PSUM, copied to bf16, and moved by the
    DMA xbar (dma_start_transpose, one call per two chunks) into
    vaug [128s, nst, 80pad] whose column 64 is 1.0 - the AV matmul then
    yields numerator (rows 0:64) and softmax denominator (row 64) at once.
  - scores St [128s, t] / exp / causal-mask / AV per s-tile i; diagonal
    tiles are narrowed to their valid t-range. exp runs per-i on ACT
    (short dependency chains).
  - ut is transposed back to [t, 65] via regular fp32 matmuls against the
    fp32 identity; rows are scaled by 1/denominator and DMA'd out with one
    DMA per chunk.
  - Emission order weaves TWO independent attention chunk-streams (so the
    in-order PE queue always has work while exp chains resolve) and
    sprinkles the next pair's projection work between their steps.
"""

import numpy as np

import concourse.bass as bass
import concourse.tile as tile
from concourse import bacc, mybir
from concourse.bass_utils import run_bass_kernel_spmd
from concourse.masks import make_identity

F32 = mybir.dt.float32
BF16 = mybir.dt.bfloat16

B, T, C, HD = 16, 2048, 1024, 64
N_CORES = 8
BL = B // N_CORES  # batches per core

P = 128
TCH = 512  # t-chunk (PSUM bank width in fp32)
VP = 80    # vaug padded inner dim (32B-aligned xbar slices)


def build_attention(ctx, tc, out, x, wk, wq, wv, b_l, t_dim, c_dim):
    nc = tc.nc
    ncc = c_dim // P        # c chunks (contraction)
    nj = t_dim // TCH       # t chunks
    ntt = TCH // P          # t subtiles per chunk
    nst = t_dim // P        # s tiles
    scale = 1.0 / float(np.sqrt(np.float32(c_dim)))

    const_pool = ctx.enter_context(tc.tile_pool(name="const", bufs=1))
    io_pool = ctx.enter_context(tc.tile_pool(name="io", bufs=1))
    big_pool = ctx.enter_context(tc.tile_pool(name="big", bufs=1))
    per_b = ctx.enter_context(tc.tile_pool(name="per_b", bufs=1))
    work = ctx.enter_context(tc.tile_pool(name="work", bufs=1))
    ps = ctx.enter_context(tc.tile_pool(name="ps", bufs=1, space="PSUM"))

    identf = const_pool.tile([P, P], F32, name="identf")
    wqk_f = const_pool.tile([P, ncc, 2 * HD], F32, name="wqk_f")
    wv_f = const_pool.tile([P, ncc, HD], F32, name="wv_f")
    wqk_sb = const_pool.tile([P, ncc, 2 * HD], BF16, name="wqk_sb")
    wv_sb = const_pool.tile([P, ncc, HD], BF16, name="wv_sb")

    masks = {}

    def weights_setup():
        # Stationary weights, cast to bf16: [c_inner=128, c_chunk, heads].
        nc.sync.dma_start(wqk_f[:, :, 0:HD], wq.rearrange("(o p) h -> p o h", p=P))
        nc.sync.dma_start(
            wqk_f[:, :, HD : 2 * HD], wk.rearrange("(o p) h -> p o h", p=P)
        )
        nc.sync.dma_start(wv_f[:], wv.rearrange("(o p) h -> p o h", p=P))
        nc.vector.tensor_copy(wqk_sb[:], wqk_f[:])
        nc.vector.tensor_copy(wv_sb[:], wv_f[:])
        make_identity(nc, identf)
        # 0/1 causal masks for diagonal s-tiles: keep f >= ss (one per width)
        mk = const_pool.tile([P, TCH], BF16, name="mask_diag")
        nc.gpsimd.memset(mk[:], 1.0)
        nc.gpsimd.affine_select(
            out=mk,
            in_=mk,
            compare_op=mybir.AluOpType.is_ge,
            fill=0.0,
            base=0,
            channel_multiplier=-1,
            pattern=[[1, TCH]],
        )
        masks[0] = mk
        # warm up the PE (HAM) during the DMA-bound startup with junk matmuls
        junk = ps.tile([P, P], F32, tag="tp", bufs=1, name="junk_warm")
        for r in range(60):
            nc.tensor.matmul(junk, identf, identf, start=True, stop=True)

    qkT = {}   # q rows 0:64, k rows 64:128
    qkT2 = {}  # k rows 0:64, q rows 64:128
    vaug = {}
    vtsb = {}

    def batch_setup(b):
        qkT[b] = per_b.tile([P, t_dim], BF16, name=f"qkT_{b}", tag="qkT", bufs=2)
        qkT2[b] = per_b.tile([P, t_dim], BF16, name=f"qkT2_{b}", tag="qkT2", bufs=2)
        vaug[b] = per_b.tile([P, nst, VP], BF16, name=f"vaug_{b}", tag="vaug", bufs=2)
        nc.gpsimd.memset(vaug[b][:, :, HD], 1.0)

    def a_phase_thunks(b, j):
        """Load/cast/transpose one 512-t chunk of x and project q/k/v.

        Returns a list of emission thunks so the weave can interleave them
        between attention steps.
        """
        st = {}
        jt = slice(j * TCH, (j + 1) * TCH)

        def load():
            xin = io_pool.tile(
                [P, ntt, c_dim], F32, tag="xin", bufs=4, name=f"xin_{b}_{j}"
            )
            nc.gpsimd.dma_start(
                xin, x[b, jt, :].bitcast(F32).rearrange("(tt p) c -> p tt c", p=P)
            )
            st["xin"] = xin

        def cast_xbar():
            xbf = io_pool.tile(
                [P, ntt, c_dim], BF16, tag="xbf", bufs=3, name=f"xbf_{b}_{j}"
            )
            nc.vector.tensor_copy(xbf, st["xin"])
            # one xbar call transposes the whole chunk into
            # xT[c_inner, tt*ncc+cc, t_lo]
            xT = big_pool.tile(
                [P, ntt * ncc, P], BF16, tag="xT", bufs=3, name=f"xT_{b}_{j}"
            )
            nc.sync.dma_start_transpose(xT, xbf)
            st["xT"] = xT
            st["qkps"] = ps.tile(
                [P, TCH], F32, tag="qkps", bufs=1, name=f"qkps_{b}_{j}"
            )
            st["vps"] = ps.tile(
                [HD, TCH], F32, tag="vps", bufs=1, name=f"vps_{b}_{j}"
            )

        def make_proj(cc0, cc1):
            def f():
                for cc in range(cc0, cc1):
                    # [128, tt, 128] strided view: columns t = tt*128 + t_lo
                    rhs = st["xT"][:, cc : ntt * ncc : ncc, :]
                    nc.tensor.matmul(
                        st["qkps"],
                        wqk_sb[:, cc, :],
                        rhs,
                        start=(cc == 0),
                        stop=(cc == ncc - 1),
                    )
                    nc.tensor.matmul(
                        st["vps"],
                        wv_sb[:, cc, :],
                        rhs,
                        start=(cc == 0),
                        stop=(cc == ncc - 1),
                    )
            return f

        def epilogue():
            nc.vector.tensor_copy(qkT[b][:, jt], st["qkps"])
            nc.vector.tensor_copy(qkT2[b][0:HD, jt], qkT[b][HD:P, jt])
            nc.vector.tensor_copy(qkT2[b][HD:P, jt], qkT[b][0:HD, jt])
            if j % 2 == 0:
                vtsb[b] = work.tile(
                    [HD, 2, TCH], BF16, tag="vtsb", bufs=2, name=f"vt_{b}_{j}"
                )
            nc.vector.tensor_copy(vtsb[b][:, j % 2, :], st["vps"])
            if j % 2 == 1:
                nc.sync.dma_start_transpose(
                    vaug[b][:, (j - 1) * ntt : (j + 1) * ntt, 0:HD], vtsb[b]
                )
            elif j == nj - 1:
                nc.sync.dma_start_transpose(
                    vaug[b][:, j * ntt : (j + 1) * ntt, 0:HD], vtsb[b][:, 0, :]
                )

        h = (ncc + 1) // 2
        return [load, cast_xbar, make_proj(0, h), make_proj(h, ncc), epilogue]

    def b_stream_thunks(b, j):
        """Attention steps for one 512-t chunk: per s-tile i score/exp/mask/AV,
        then the finale (transpose back, divide, store)."""
        st = {}
        ni = ntt * j + ntt
        jt0 = j * TCH

        def make_pair(i0):
            def f():
                if i0 == 0:
                    st["utps"] = ps.tile(
                        [HD + 1, TCH], F32, tag="utps", bufs=2, name=f"ut_{b}_{j}"
                    )
                n_pair = min(2, ni - i0)
                sub = []
                for di in range(n_pair):
                    i = i0 + di
                    diag = i >= ntt * j
                    w0 = (i - ntt * j) * P if diag else 0
                    sub.append((i, diag, w0, TCH - w0))
                # score matmuls back-to-back: even/odd i use PE row groups
                # 0/1, so adjacent issues execute concurrently
                sts = []
                for (i, diag, w0, wn) in sub:
                    stps = ps.tile(
                        [P, TCH], F32, tag="stq", bufs=3, name=f"st_{b}_{j}_{i}"
                    )
                    lo, hi = (0, HD) if i % 2 == 0 else (HD, P)
                    lhs = qkT[b] if i % 2 == 0 else qkT2[b]
                    rhs = qkT2[b] if i % 2 == 0 else qkT[b]
                    nc.tensor.matmul(
                        stps[:, 0:wn],
                        lhs[lo:hi, i * P : (i + 1) * P],
                        rhs[lo:hi, jt0 + w0 : jt0 + TCH],
                        start=True,
                        stop=True,
                    )
                    sts.append(stps)
                pts = []
                for (i, diag, w0, wn), stps in zip(sub, sts):
                    pt = work.tile(
                        [P, TCH], BF16, tag="pt", bufs=6, name=f"pt_{b}_{j}_{i}"
                    )
                    nc.scalar.activation(
                        pt[:, 0:wn], stps[:, 0:wn],
                        mybir.ActivationFunctionType.Exp, scale=scale,
                    )
                    if diag:
                        # causal mask: multiply by the 0/1 triangle (keep
                        # f >= ss); cheaper on the exp->AV chain than an
                        # affine_select on gpsimd
                        nc.vector.tensor_mul(
                            pt[:, 0:wn], pt[:, 0:wn], masks[0][:, 0:wn]
                        )
                    pts.append(pt)
                for (i, diag, w0, wn), pt in zip(sub, pts):
                    nc.tensor.matmul(
                        st["utps"][:, w0:TCH],
                        vaug[b][:, i, 0 : HD + 1],
                        pt[:, 0:wn],
                        start=(i == 0),
                        stop=(i == ni - 1),
                        skip_group_check=True,
                    )
            return f

        def finale():
            utsb = work.tile([P, TCH], F32, tag="utsb", bufs=2, name=f"utsb_{b}_{j}")
            nc.any.memzero(utsb[HD:P, :])  # garbage rows would NaN the transpose
            nc.vector.tensor_copy(utsb[0 : HD + 1, :], st["utps"])
            otp = ps.tile([P, TCH], F32, tag="tp", bufs=1, name=f"otp_{b}_{j}")
            for tt in range(ntt):
                nc.tensor.matmul(
                    otp[:, tt * P : (tt + 1) * P],
                    utsb[:, tt * P : (tt + 1) * P],
                    identf,
                    start=True,
                    stop=True,
                )
            rec = work.tile([P, ntt], F32, tag="rec", bufs=2, name=f"rec_{b}_{j}")
            nc.vector.reciprocal(rec, otp[:, HD::P])
            osb = io_pool.tile([P, ntt, HD], F32, tag="osb", bufs=2, name=f"osb_{b}_{j}")
            for tt in range(ntt):
                nc.vector.tensor_scalar_mul(
                    osb[:, tt, :], otp[:, tt * P : tt * P + HD], rec[:, tt : tt + 1]
                )
            nc.sync.dma_start(
                out[b, j * TCH : (j + 1) * TCH, :].rearrange(
                    "(tt p) h -> p tt h", p=P
                ),
                osb,
            )

        return [make_pair(i0) for i0 in range(0, ni, 2)] + [finale]

    def weave(streams, inject):
        nsteps = max(len(s) for s in streams)
        ai, na = 0, len(inject)
        for step in range(nsteps):
            for s in streams:
                if step < len(s):
                    s[step]()
            tgt = (step + 1) * na // nsteps
            while ai < tgt:
                inject[ai]()
                ai += 1

    # chunk pairs per batch; A-phase work for pair k+1 is woven into pair k
    pairs = []
    for b in range(b_l):
        js = list(range(nj))
        pairs += [(b, tuple(js[k : k + 2])) for k in range(0, nj, 2)]
    a_flat = [(b, j) for b, jl in pairs for j in jl]

    def interleaved(chunk_thunks):
        # x-load DMAs first (issue ahead), then the dependent work
        return [c[0] for c in chunk_thunks] + [t for c in chunk_thunks for t in c[1:]]

    n0 = len(pairs[0][1])
    eager = [a_phase_thunks(b, j) for (b, j) in a_flat[:n0]]
    for c in eager:  # x DMAs for the first pair go out before anything else
        c[0]()
    weights_setup()
    for b in range(b_l):
        batch_setup(b)
    for c in eager:
        for t in c[1:]:
            t()
    ai = n0
    for k, (b, jl) in enumerate(pairs):
        nxt = a_flat[ai : ai + (len(pairs[k + 1][1]) if k + 1 < len(pairs) else 0)]
        ai += len(nxt)
        inject = interleaved([a_phase_thunks(bb, jj) for (bb, jj) in nxt])
        weave([b_stream_thunks(b, j) for j in jl], inject)


def build_nc(b_l=BL, t_dim=T, c_dim=C):
    nc = bacc.Bacc("TRN2", target_bir_lowering=False, debug=False)
    # x is bound as a bf16 VIEW of the fp32 input buffer: element 2c+1 is
    # the high halfword of x[..., c], i.e. bf16(x) by truncation
    x = nc.dram_tensor("x", [b_l, t_dim, 2 * c_dim], BF16, kind="ExternalInput").ap()
    wk = nc.dram_tensor("Wk", [c_dim, HD], F32, kind="ExternalInput").ap()
    wq = nc.dram_tensor("Wq", [c_dim, HD], F32, kind="ExternalInput").ap()
    wv = nc.dram_tensor("Wv", [c_dim, HD], F32, kind="ExternalInput").ap()
    out = nc.dram_tensor("out", [b_l, t_dim, HD], F32, kind="ExternalOutput").ap()
    from contextlib import ExitStack

    with tile.TileContext(nc) as tc, ExitStack() as ctx:
        build_attention(ctx, tc, out, x, wk, wq, wv, b_l, t_dim, c_dim)
    nc.compile()
    return nc


_NC_CACHE = {}


def _get_nc():
    if "nc" not in _NC_CACHE:
        _NC_CACHE["nc"] = build_nc()
    return _NC_CACHE["nc"]


def kernel(x, Wk, Wq, Wv, _trace=False, _tmpdir=None):
    import ml_dtypes

    x = np.ascontiguousarray(np.asarray(x, dtype=np.float32))
    Wk = np.ascontiguousarray(np.asarray(Wk, dtype=np.float32))
    Wq = np.ascontiguousarray(np.asarray(Wq, dtype=np.float32))
    Wv = np.ascontiguousarray(np.asarray(Wv, dtype=np.float32))
    # bind x as a bf16 view (kernel reads the high halfword of each fp32)
    xv = x.view(ml_dtypes.bfloat16)
    nc = _get_nc()
    in_maps = [
        {"x": xv[c * BL : (c + 1) * BL], "Wk": Wk, "Wq": Wq, "Wv": Wv}
        for c in range(N_CORES)
    ]
    res = run_bass_kernel_spmd(
        nc, in_maps, core_ids=list(range(N_CORES)), trace=_trace, tmpdir=_tmpdir
    )
    out = np.concatenate([res.results[c]["out"] for c in range(N_CORES)], axis=0)
    if _trace:
        kernel.last_exec_time_ns = res.exec_time_ns
        kernel.last_results = res
    return out


# revision 54
# speedup vs baseline: 1.1733x; 1.1733x over previous
"""Trainium2 Bass kernel: single-head causal attention (B=16, T=2048, C=1024, HD=64).

Data-parallel over batch across 8 NeuronCores (2 batches/core), weights
replicated. Each core computes, per batch:
    q = x @ Wq, k = x @ Wk, v = x @ Wv            (via transposed layouts)
    scores[t, s] = k[t] . q[s] / sqrt(C)          (computed transposed: St[s, t])
    causal mask (keep s <= t), softmax over s, out[t] = sum_s w[t, s] v[s]

Compute dtype is bf16 on the TensorEngine (PSUM accumulation in fp32, final
softmax division in fp32); rel-err vs the fp32 reference is ~4e-3.

Layout strategy (v3):
  - x arrives as one DMA per 512-t chunk [128, 4tt, 1024c], is cast to bf16
    (DVE), then transposed 128x128-at-a-time on the PE as REGULAR bf16
    matmuls against a bf16 identity (LDW = x tile, stream identity, N=128) -
    much faster than transpose-mode; 4 tiles pack into one PSUM bank and one
    DVE copy moves them to xT [128 c_inner, ncc, t].
  - q/k come from one stacked [Wq|Wk] projection into qkT (q rows 0:64, k
    rows 64:128); a swapped duplicate qkT2 (k lo, q hi) lets the K=64 score
    matmuls run two-at-a-time in separate PE row groups.
  - v is projected into vT [64h, s] PSUM, copied to bf16, and moved by the
    DMA xbar (dma_start_transpose, one call per two chunks) into
    vaug [128s, nst, 80pad] whose column 64 is 1.0 - the AV matmul then
    yields numerator (rows 0:64) and softmax denominator (row 64) at once.
  - scores St [128s, t] / exp / causal-mask / AV per s-tile i; diagonal
    tiles are narrowed to their valid t-range. exp runs per-i on ACT
    (short dependency chains).
  - ut is transposed back to [t, 65] via regular fp32 matmuls against the
    fp32 identity; rows are scaled by 1/denominator and DMA'd out with one
    DMA per chunk.
  - Emission order weaves TWO independent attention chunk-streams (so the
    in-order PE queue always has work while exp chains resolve) and
    sprinkles the next pair's projection work between their steps.
"""

import numpy as np

import concourse.bass as bass
import concourse.tile as tile
from concourse import bacc, mybir
from concourse.bass_utils import run_bass_kernel_spmd
from concourse.masks import make_identity

F32 = mybir.dt.float32
BF16 = mybir.dt.bfloat16

B, T, C, HD = 16, 2048, 1024, 64
N_CORES = 8
BL = B // N_CORES  # batches per core

P = 128
TCH = 512  # t-chunk (PSUM bank width in fp32)
VP = 80    # vaug padded inner dim (32B-aligned xbar slices)


def build_attention(ctx, tc, out, x, wk, wq, wv, b_l, t_dim, c_dim):
    nc = tc.nc
    ncc = c_dim // P        # c chunks (contraction)
    nj = t_dim // TCH       # t chunks
    ntt = TCH // P          # t subtiles per chunk
    nst = t_dim // P        # s tiles
    scale = 1.0 / float(np.sqrt(np.float32(c_dim)))

    const_pool = ctx.enter_context(tc.tile_pool(name="const", bufs=1))
    io_pool = ctx.enter_context(tc.tile_pool(name="io", bufs=1))
    big_pool = ctx.enter_context(tc.tile_pool(name="big", bufs=1))
    per_b = ctx.enter_context(tc.tile_pool(name="per_b", bufs=1))
    work = ctx.enter_context(tc.tile_pool(name="work", bufs=1))
    ps = ctx.enter_context(tc.tile_pool(name="ps", bufs=1, space="PSUM"))

    identf = const_pool.tile([P, P], F32, name="identf")
    wqk_f = const_pool.tile([P, ncc, 2 * HD], F32, name="wqk_f")
    wv_f = const_pool.tile([P, ncc, HD], F32, name="wv_f")
    wqk_sb = const_pool.tile([P, ncc, 2 * HD], BF16, name="wqk_sb")
    wv_sb = const_pool.tile([P, ncc, HD], BF16, name="wv_sb")

    masks = {}

    def weights_setup():
        # Stationary weights, cast to bf16: [c_inner=128, c_chunk, heads].
        nc.sync.dma_start(wqk_f[:, :, 0:HD], wq.rearrange("(o p) h -> p o h", p=P))
        nc.sync.dma_start(
            wqk_f[:, :, HD : 2 * HD], wk.rearrange("(o p) h -> p o h", p=P)
        )
        nc.sync.dma_start(wv_f[:], wv.rearrange("(o p) h -> p o h", p=P))
        nc.vector.tensor_copy(wqk_sb[:], wqk_f[:])
        nc.vector.tensor_copy(wv_sb[:], wv_f[:])
        make_identity(nc, identf)
        # 0/1 causal masks for diagonal s-tiles: keep f >= ss (one per width)
        mk = const_pool.tile([P, TCH], BF16, name="mask_diag")
        nc.gpsimd.memset(mk[:], 1.0)
        nc.gpsimd.affine_select(
            out=mk,
            in_=mk,
            compare_op=mybir.AluOpType.is_ge,
            fill=0.0,
            base=0,
            channel_multiplier=-1,
            pattern=[[1, TCH]],
        )
        masks[0] = mk
        # warm up the PE (HAM) during the DMA-bound startup with junk matmuls
        junk = ps.tile([P, P], F32, tag="tp", bufs=1, name="junk_warm")
        for r in range(60):
            nc.tensor.matmul(junk, identf, identf, start=True, stop=True)

    qkT = {}   # q rows 0:64, k rows 64:128
    qkT2 = {}  # k rows 0:64, q rows 64:128
    vaug = {}
    vtsb = {}

    def batch_setup(b):
        qkT[b] = per_b.tile([P, t_dim], BF16, name=f"qkT_{b}", tag="qkT", bufs=2)
        qkT2[b] = per_b.tile([P, t_dim], BF16, name=f"qkT2_{b}", tag="qkT2", bufs=2)
        vaug[b] = per_b.tile([P, nst, VP], BF16, name=f"vaug_{b}", tag="vaug", bufs=2)
        nc.gpsimd.memset(vaug[b][:, :, HD], 1.0)

    def a_phase_thunks(b, j):
        """Load/cast/transpose one 512-t chunk of x and project q/k/v.

        Returns a list of emission thunks so the weave can interleave them
        between attention steps.
        """
        st = {}
        jt = slice(j * TCH, (j + 1) * TCH)

        def load():
            xin = io_pool.tile(
                [P, ntt, c_dim], F32, tag="xin", bufs=4, name=f"xin_{b}_{j}"
            )
            nc.sync.dma_start(
                xin, x[b, jt, :].bitcast(F32).rearrange("(tt p) c -> p tt c", p=P)
            )
            st["xin"] = xin

        def cast_xbar():
            xbf = io_pool.tile(
                [P, ntt, c_dim], BF16, tag="xbf", bufs=3, name=f"xbf_{b}_{j}"
            )
            nc.vector.tensor_copy(xbf, st["xin"])
            # one xbar call transposes the whole chunk into
            # xT[c_inner, tt*ncc+cc, t_lo]
            xT = big_pool.tile(
                [P, ntt * ncc, P], BF16, tag="xT", bufs=3, name=f"xT_{b}_{j}"
            )
            nc.sync.dma_start_transpose(xT, xbf)
            st["xT"] = xT
            st["qkps"] = ps.tile(
                [P, TCH], F32, tag="qkps", bufs=1, name=f"qkps_{b}_{j}"
            )
            st["vps"] = ps.tile(
                [HD, TCH], F32, tag="vps", bufs=1, name=f"vps_{b}_{j}"
            )

        def make_proj(cc0, cc1):
            def f():
                for cc in range(cc0, cc1):
                    # [128, tt, 128] strided view: columns t = tt*128 + t_lo
                    rhs = st["xT"][:, cc : ntt * ncc : ncc, :]
                    nc.tensor.matmul(
                        st["qkps"],
                        wqk_sb[:, cc, :],
                        rhs,
                        start=(cc == 0),
                        stop=(cc == ncc - 1),
                    )
                    nc.tensor.matmul(
                        st["vps"],
                        wv_sb[:, cc, :],
                        rhs,
                        start=(cc == 0),
                        stop=(cc == ncc - 1),
                    )
            return f

        def epilogue():
            nc.vector.tensor_copy(qkT[b][:, jt], st["qkps"])
            nc.vector.tensor_copy(qkT2[b][0:HD, jt], qkT[b][HD:P, jt])
            nc.vector.tensor_copy(qkT2[b][HD:P, jt], qkT[b][0:HD, jt])
            if j % 2 == 0:
                vtsb[b] = work.tile(
                    [HD, 2, TCH], BF16, tag="vtsb", bufs=2, name=f"vt_{b}_{j}"
                )
            nc.vector.tensor_copy(vtsb[b][:, j % 2, :], st["vps"])
            if j % 2 == 1:
                nc.sync.dma_start_transpose(
                    vaug[b][:, (j - 1) * ntt : (j + 1) * ntt, 0:HD], vtsb[b]
                )
            elif j == nj - 1:
                nc.sync.dma_start_transpose(
                    vaug[b][:, j * ntt : (j + 1) * ntt, 0:HD], vtsb[b][:, 0, :]
                )

        h = (ncc + 1) // 2
        return [load, cast_xbar, make_proj(0, h), make_proj(h, ncc), epilogue]

    def b_stream_thunks(b, j):
        """Attention steps for one 512-t chunk: per s-tile i score/exp/mask/AV,
        then the finale (transpose back, divide, store)."""
        st = {}
        ni = ntt * j + ntt
        jt0 = j * TCH

        def make_pair(i0):
            def f():
                if i0 == 0:
                    st["utps"] = ps.tile(
                        [HD + 1, TCH], F32, tag="utps", bufs=2, name=f"ut_{b}_{j}"
                    )
                n_pair = min(2, ni - i0)
                sub = []
                for di in range(n_pair):
                    i = i0 + di
                    diag = i >= ntt * j
                    w0 = (i - ntt * j) * P if diag else 0
                    sub.append((i, diag, w0, TCH - w0))
                # score matmuls back-to-back: even/odd i use PE row groups
                # 0/1, so adjacent issues execute concurrently
                sts = []
                for (i, diag, w0, wn) in sub:
                    stps = ps.tile(
                        [P, TCH], F32, tag="stq", bufs=3, name=f"st_{b}_{j}_{i}"
                    )
                    lo, hi = (0, HD) if i % 2 == 0 else (HD, P)
                    lhs = qkT[b] if i % 2 == 0 else qkT2[b]
                    rhs = qkT2[b] if i % 2 == 0 else qkT[b]
                    nc.tensor.matmul(
                        stps[:, 0:wn],
                        lhs[lo:hi, i * P : (i + 1) * P],
                        rhs[lo:hi, jt0 + w0 : jt0 + TCH],
                        start=True,
                        stop=True,
                    )
                    sts.append(stps)
                pts = []
                for (i, diag, w0, wn), stps in zip(sub, sts):
                    pt = work.tile(
                        [P, TCH], BF16, tag="pt", bufs=6, name=f"pt_{b}_{j}_{i}"
                    )
                    nc.scalar.activation(
                        pt[:, 0:wn], stps[:, 0:wn],
                        mybir.ActivationFunctionType.Exp, scale=scale,
                    )
                    if diag:
                        # causal mask: multiply by the 0/1 triangle (keep
                        # f >= ss); cheaper on the exp->AV chain than an
                        # affine_select on gpsimd
                        nc.vector.tensor_mul(
                            pt[:, 0:wn], pt[:, 0:wn], masks[0][:, 0:wn]
                        )
                    pts.append(pt)
                for (i, diag, w0, wn), pt in zip(sub, pts):
                    nc.tensor.matmul(
                        st["utps"][:, w0:TCH],
                        vaug[b][:, i, 0 : HD + 1],
                        pt[:, 0:wn],
                        start=(i == 0),
                        stop=(i == ni - 1),
                        skip_group_check=True,
                    )
            return f

        def finale():
            utsb = work.tile([P, TCH], F32, tag="utsb", bufs=2, name=f"utsb_{b}_{j}")
            nc.any.memzero(utsb[HD:P, :])  # garbage rows would NaN the transpose
            nc.vector.tensor_copy(utsb[0 : HD + 1, :], st["utps"])
            otp = ps.tile([P, TCH], F32, tag="tp", bufs=1, name=f"otp_{b}_{j}")
            for tt in range(ntt):
                nc.tensor.matmul(
                    otp[:, tt * P : (tt + 1) * P],
                    utsb[:, tt * P : (tt + 1) * P],
                    identf,
                    start=True,
                    stop=True,
                )
            rec = work.tile([P, ntt], F32, tag="rec", bufs=2, name=f"rec_{b}_{j}")
            nc.vector.reciprocal(rec, otp[:, HD::P])
            osb = io_pool.tile([P, ntt, HD], F32, tag="osb", bufs=2, name=f"osb_{b}_{j}")
            for tt in range(ntt):
                nc.vector.tensor_scalar_mul(
                    osb[:, tt, :], otp[:, tt * P : tt * P + HD], rec[:, tt : tt + 1]
                )
            nc.sync.dma_start(
                out[b, j * TCH : (j + 1) * TCH, :].rearrange(
                    "(tt p) h -> p tt h", p=P
                ),
                osb,
            )

        return [make_pair(i0) for i0 in range(0, ni, 2)] + [finale]

    def weave(streams, inject):
        nsteps = max(len(s) for s in streams)
        ai, na = 0, len(inject)
        for step in range(nsteps):
            for s in streams:
                if step < len(s):
                    s[step]()
            tgt = (step + 1) * na // nsteps
            while ai < tgt:
                inject[ai]()
                ai += 1

    # chunk pairs per batch; A-phase work for pair k+1 is woven into pair k
    pairs = []
    for b in range(b_l):
        js = list(range(nj))
        pairs += [(b, tuple(js[k : k + 2])) for k in range(0, nj, 2)]
    a_flat = [(b, j) for b, jl in pairs for j in jl]

    def interleaved(chunk_thunks):
        # x-load DMAs first (issue ahead), then the dependent work
        return [c[0] for c in chunk_thunks] + [t for c in chunk_thunks for t in c[1:]]

    n0 = len(pairs[0][1])
    eager = [a_phase_thunks(b, j) for (b, j) in a_flat[:n0]]
    for c in eager:  # x DMAs for the first pair go out before anything else
        c[0]()
    weights_setup()
    for b in range(b_l):
        batch_setup(b)
    for c in eager:
        for t in c[1:]:
            t()
    ai = n0
    for k, (b, jl) in enumerate(pairs):
        nxt = a_flat[ai : ai + (len(pairs[k + 1][1]) if k + 1 < len(pairs) else 0)]
        ai += len(nxt)
        inject = interleaved([a_phase_thunks(bb, jj) for (bb, jj) in nxt])
        weave([b_stream_thunks(b, j) for j in jl], inject)


def build_nc(b_l=BL, t_dim=T, c_dim=C):
    nc = bacc.Bacc("TRN2", target_bir_lowering=False, debug=False)
    # x is bound as a bf16 VIEW of the fp32 input buffer: element 2c+1 is
    # the high halfword of x[..., c], i.e. bf16(x) by truncation
    x = nc.dram_tensor("x", [b_l, t_dim, 2 * c_dim], BF16, kind="ExternalInput").ap()
    wk = nc.dram_tensor("Wk", [c_dim, HD], F32, kind="ExternalInput").ap()
    wq = nc.dram_tensor("Wq", [c_dim, HD], F32, kind="ExternalInput").ap()
    wv = nc.dram_tensor("Wv", [c_dim, HD], F32, kind="ExternalInput").ap()
    out = nc.dram_tensor("out", [b_l, t_dim, HD], F32, kind="ExternalOutput").ap()
    from contextlib import ExitStack

    with tile.TileContext(nc) as tc, ExitStack() as ctx:
        build_attention(ctx, tc, out, x, wk, wq, wv, b_l, t_dim, c_dim)
    nc.compile()
    return nc


_NC_CACHE = {}


def _get_nc():
    if "nc" not in _NC_CACHE:
        _NC_CACHE["nc"] = build_nc()
    return _NC_CACHE["nc"]


def kernel(x, Wk, Wq, Wv, _trace=False, _tmpdir=None):
    import ml_dtypes

    x = np.ascontiguousarray(np.asarray(x, dtype=np.float32))
    Wk = np.ascontiguousarray(np.asarray(Wk, dtype=np.float32))
    Wq = np.ascontiguousarray(np.asarray(Wq, dtype=np.float32))
    Wv = np.ascontiguousarray(np.asarray(Wv, dtype=np.float32))
    # bind x as a bf16 view (kernel reads the high halfword of each fp32)
    xv = x.view(ml_dtypes.bfloat16)
    nc = _get_nc()
    in_maps = [
        {"x": xv[c * BL : (c + 1) * BL], "Wk": Wk, "Wq": Wq, "Wv": Wv}
        for c in range(N_CORES)
    ]
    res = run_bass_kernel_spmd(
        nc, in_maps, core_ids=list(range(N_CORES)), trace=_trace, tmpdir=_tmpdir
    )
    out = np.concatenate([res.results[c]["out"] for c in range(N_CORES)], axis=0)
    if _trace:
        kernel.last_exec_time_ns = res.exec_time_ns
        kernel.last_results = res
    return out
